# revision 28
# baseline (speedup 1.0000x reference)
"""Trainium2 Bass kernel for top-2 MoE routing (nn_JaxMoE_26431228740246).

Strategy: F-parallel over all experts across 8 NeuronCores with host-side
routing/dispatch (the standard MoE dispatch/combine).  The router is
T*D*E = 16.8M MACs -- 0.008% of total FLOPs -- and determines the sharding,
so it runs on host.  Tokens are gathered per expert (token t appears in the
segments of its top-2 experts; Sum n_e = T*K = 4096 exactly); every core
receives the SAME dispatched token buffer but only an F/8 = 512 slice of
every expert's gate/up/down weights.  Each core computes, for each expert
segment, h = silu(Wg_slice.T x)*(Wu_slice.T x) and the partial down
projection y_c = Wd_slice.T h.  Host combine sums the 8 partial outputs and
scatter-adds with the renormalized router weights.

This is perfectly load balanced (all cores do identical 96*4096 = 393k PE
cycles -- the exact useful-FLOP floor at full-rate matmul) with zero
capacity padding, because bf16 matmuls run 1 cycle/row at any free-dim
width.

Everything on-device is bf16 (inputs quantized on host; PSUM accumulation is
fp32; partial outputs returned bf16 and summed in fp32 on host) -- measured
end-to-end error ~6e-4 against the fp32 reference, tolerance is 2e-2.

DMA layout notes: all tiles are host-pre-tiled so each DMA moves multi-KB
contiguous per-partition lines.  Gate/up weights + x stream on the sync
HWDGE queue; down weights + outputs use the scalar queue (separate FIFO +
second descriptor generator) so they prefetch without head-of-line blocking.

Shapes (hardcoded): T=2048, D=1024, F=4096, E=8, K=2 (top-k renormalized).
"""

import os
import sys

import numpy as np


def _ensure_path():
    for p in (
        "/root/.axon_site",
        "/root/.axon_site/_ro/trn_rl_repo",
        "/root/.axon_site/_ro/pypackages",
        "/opt/trn_rl_repo",
    ):
        if os.path.isdir(p) and p not in sys.path:
            sys.path.append(p)


_ensure_path()

T, D, F, E = 2048, 1024, 4096, 8
DT = D // 128    # 8 d-tiles
FS = F // E      # 512 = per-core F slice
FST = FS // 128  # 4 f-tiles per expert per core

_CACHE = {}


def _chunks(n):
    """Split a segment of n tokens into <=512-wide PSUM chunks (even halves)."""
    if n <= 512:
        return [(0, n)]
    h = (n + 1) // 2
    return [(0, h), (h, n)]


def _build(counts):
    import concourse.tile as tile
    from concourse import bacc, mybir

    fp32 = mybir.dt.float32
    bf16 = mybir.dt.bfloat16
    Act = mybir.ActivationFunctionType

    counts = list(counts)
    N = sum(counts)
    off = np.concatenate([[0], np.cumsum(counts)]).astype(int)

    nc = bacc.Bacc("TRN2", target_bir_lowering=False, debug=False, num_devices=E)

    # x dispatched, transposed, per-expert-contiguous: [128(di), sum_e 8*n_e]
    xT_d = nc.dram_tensor("xT", [128, DT * N], bf16, kind="ExternalInput").ap()
    # combined weight tile per (expert, f-tile): [128, {wg(do,f)|wu(do,f)|wd(dt,dc)}]
    # wd rides the gate-weight stream so its DMAs are paced by the pool
    # rotation instead of being hoisted to t=0 by the scheduler.
    w_d = nc.dram_tensor(
        "w", [E, FST, 128, 3, DT, 128], bf16, kind="ExternalInput"
    ).ap()
    # partial output: per expert a [128, DT, n_e] block, flattened columns
    out_d = nc.dram_tensor("out", [128, DT * N], bf16, kind="ExternalOutput").ap()

    from contextlib import ExitStack

    with tile.TileContext(nc) as tc, ExitStack() as ctx:
        px = ctx.enter_context(tc.tile_pool(name="x", bufs=1))
        pw = ctx.enter_context(tc.tile_pool(name="w", bufs=9))
        ph = ctx.enter_context(tc.tile_pool(name="h", bufs=1))
        ptmp = ctx.enter_context(tc.tile_pool(name="tmp", bufs=4))
        pout = ctx.enter_context(tc.tile_pool(name="out", bufs=3))
        pmm = ctx.enter_context(tc.tile_pool(name="mm", bufs=8, space="PSUM"))

        # ---- per expert: gate/up -> h, then partial down-projection ----
        # (down(e) overlaps gate(e+1) weight streaming; out DMAs spread
        # across the whole kernel instead of bursting at the end)
        for e in range(E):
            n = counts[e]
            ch = _chunks(n)
            xsrc = xT_d[:, DT * off[e] : DT * off[e + 1]].rearrange(
                "p (do n) -> p do n", do=DT
            )
            if e == 0:
                # separate per-do tiles: dependency tracking is per-tile, so
                # the first accumulation chain streams in do-granular pieces
                xe = [px.tile([128, n], bf16, tag=f"x0_{do}", name=f"x0_{do}") for do in range(DT)]
            else:
                xe = px.tile([128, DT, n], bf16, tag=f"x{e}")
            he = ph.tile([128, FST, n], bf16, tag=f"h{e}")

            def xop(do, c0, c1):
                if e == 0:
                    return xe[do][:, c0:c1]
                return xe[:, do, c0:c1]

            # gate/up phase for this expert
            wts = []
            for ft in range(FST):
                wt = pw.tile([128, 3, DT, 128], bf16, tag="w", name=f"w{e}_{ft}")
                if e == 0 and ft == 0:
                    # split the very first weight DMA into its three parts so
                    # the first chain's stationary operand lands sooner
                    for part in range(3):
                        nc.sync.dma_start(wt[:, part], w_d[e, ft, :, part])
                else:
                    nc.sync.dma_start(wt[:], w_d[e, ft])
                wts.append(wt)
                if ft == 0:
                    # x DMA after the first weight tiles; for e=0 split it
                    # per-do (on the otherwise idle vector queue) so the
                    # first accumulation chain streams without serializing
                    # the sync SEQ
                    if e == 0:
                        for do in range(DT):
                            nc.sync.dma_start(xe[do][:], xsrc[:, do, :])
                    else:
                        nc.sync.dma_start(xe[:], xsrc)

                pgs = [pmm.tile([128, c1 - c0], fp32, tag="mm", name="pg") for c0, c1 in ch]
                pus = [pmm.tile([128, c1 - c0], fp32, tag="mm", name="pu") for c0, c1 in ch]
                for (c0, c1), pg, pu in zip(ch, pgs, pus):
                    for do in range(DT):
                        nc.tensor.matmul(
                            pg[:], wt[:, 0, do, :], xop(do, c0, c1),
                            start=(do == 0), stop=(do == DT - 1),
                        )
                    for do in range(DT):
                        nc.tensor.matmul(
                            pu[:], wt[:, 1, do, :], xop(do, c0, c1),
                            start=(do == 0), stop=(do == DT - 1),
                        )
                    tmp = ptmp.tile([128, c1 - c0], fp32, tag="tmp")
                    nc.scalar.activation(tmp[:], pg[:], Act.Silu)
                    nc.vector.tensor_mul(he[:, ft, c0:c1], tmp[:], pu[:])

            # down phase for this expert (wd slice fo=ft lives in wts[ft])
            ot = pout.tile([128, DT, n], bf16, tag="ot")
            for dd in range(DT):
                for ci, (c0, c1) in enumerate(ch):
                    po = pmm.tile([128, c1 - c0], fp32, tag="mm")
                    for fo in range(FST):
                        nc.tensor.matmul(
                            po[:], wts[fo][:, 2, dd, :],
                            he[:, fo, c0:c1],
                            start=(fo == 0), stop=(fo == FST - 1),
                        )
                    # alternate copy engine: DVE / Activation (last expert:
                    # DVE only, so the Act SEQ stays free for tail DMAs)
                    if e == E - 1 or (dd * len(ch) + ci) % 2 == 0:
                        nc.vector.tensor_copy(ot[:, dd, c0:c1], po[:])
                    else:
                        nc.scalar.copy(ot[:, dd, c0:c1], po[:])
                if dd == DT // 2 - 1:
                    nc.scalar.dma_start(
                        out_d[:, DT * off[e] : DT * off[e] + (DT // 2) * n],
                        ot[:, 0 : DT // 2, :],
                    )
                elif e == E - 1 and dd >= DT // 2:
                    # last expert: ship each dd as soon as it's copied,
                    # spread across queues so descriptor generation overlaps
                    # and the final drain is one small DMA on an idle queue
                    q = [nc.scalar, nc.sync, nc.scalar, nc.sync][dd - DT // 2]
                    q.dma_start(
                        out_d[:, DT * off[e] + dd * n : DT * off[e] + (dd + 1) * n],
                        ot[:, dd, :],
                    )
            if e != E - 1:
                nc.scalar.dma_start(
                    out_d[:, DT * off[e] + (DT // 2) * n : DT * off[e + 1]],
                    ot[:, DT // 2 : DT, :],
                )

    nc.compile()
    return nc


def _get_nc(counts):
    key = tuple(counts)
    if key not in _CACHE:
        _CACHE[key] = _build(key)
    return _CACHE[key]


def _route(x, wr):
    """Exact top-2 routing in fp64 (verified: gap between 2nd/3rd router
    logit is ~5e-4 on this data, far above fp32 matmul noise, so fp64
    ordering equals the reference's fp32 ordering)."""
    lg = x.astype(np.float64) @ wr.astype(np.float64)           # [T, E]
    top2 = np.argpartition(-lg, 2, axis=1)[:, :2]               # unordered top-2
    l2 = np.take_along_axis(lg, top2, axis=1)                   # [T, 2]
    m = l2.max(axis=1, keepdims=True)
    p = np.exp(l2 - m)
    w2 = p / p.sum(axis=1, keepdims=True)                       # renormalized
    return top2, w2


def kernel(
    x_TD, w_router_DE, kernel_gating_EDF, kernel_up_proj_EDF, kernel_down_proj_EFD
):
    import ml_dtypes
    from concourse.bass_utils import run_bass_kernel_spmd

    bf = ml_dtypes.bfloat16

    x = np.ascontiguousarray(np.asarray(x_TD, dtype=np.float32))
    wr = np.ascontiguousarray(np.asarray(w_router_DE, dtype=np.float32))
    g = np.asarray(kernel_gating_EDF, dtype=np.float32)
    u = np.asarray(kernel_up_proj_EDF, dtype=np.float32)
    d = np.asarray(kernel_down_proj_EFD, dtype=np.float32)

    # ---- host routing / dispatch ----
    top2, w2 = _route(x, wr)
    idx_list, wgt_list = [], []
    for e in range(E):
        sel = top2 == e                                         # [T, 2]
        m = sel.any(axis=1)
        tok = np.nonzero(m)[0]
        idx_list.append(tok)
        wgt_list.append(w2[m][sel[m]].astype(np.float64))
    counts = [len(i) for i in idx_list]
    N = sum(counts)
    off = np.concatenate([[0], np.cumsum(counts)]).astype(int)

    # dispatched tokens, transposed, per-expert contiguous [128, sum_e 8*n_e]
    xT_host = np.empty((128, DT * N), dtype=bf)
    for e in range(E):
        seg = x[idx_list[e]].T.reshape(DT, 128, counts[e]).transpose(1, 0, 2)
        xT_host[:, DT * off[e] : DT * off[e + 1]] = seg.reshape(
            128, DT * counts[e]
        ).astype(bf)

    nc = _get_nc(counts)

    # per-core weight slices (core c takes F columns [c*512, (c+1)*512))
    in_maps = []
    for c in range(E):
        fs = slice(c * FS, (c + 1) * FS)
        # [E, D, FS] -> [E, FST, 128di, DT, 128f]
        wg_host = g[:, :, fs].reshape(E, DT, 128, FST, 128).transpose(0, 3, 2, 1, 4)
        wu_host = u[:, :, fs].reshape(E, DT, 128, FST, 128).transpose(0, 3, 2, 1, 4)
        # [E, FS, D] -> [E, FST, 128fi, DT, 128dc]
        wd_host = d[:, fs, :].reshape(E, FST, 128, DT, 128)
        w_host = np.ascontiguousarray(
            np.stack([wg_host, wu_host, wd_host], axis=3)
        ).astype(bf)                       # [E, FST, 128, 3, DT, 128]
        in_maps.append({"xT": xT_host, "w": w_host})

    trace = bool(os.environ.get("BASS_PROF"))
    try:
        res = run_bass_kernel_spmd(nc, in_maps, list(range(E)), trace=trace)
    except Exception:
        if not trace:
            raise
        res = run_bass_kernel_spmd(nc, in_maps, list(range(E)), trace=False)
    _CACHE["last_result"] = res
    _CACHE["last_counts"] = counts

    # ---- host combine: sum partial outputs, scatter-add with router weights ----
    ysum = np.zeros((128, DT * N), dtype=np.float32)
    for c in range(E):
        ysum += np.asarray(res.results[c]["out"], dtype=np.float32).reshape(
            128, DT * N
        )
    out = np.zeros((T, D), dtype=np.float64)
    for e in range(E):
        n = counts[e]
        blk = ysum[:, DT * off[e] : DT * off[e + 1]].reshape(128, DT, n)
        y = blk.transpose(1, 0, 2).reshape(D, n)                # [D, n]
        out[idx_list[e]] += wgt_list[e][:, None] * y.T
    return np.ascontiguousarray(out.astype(np.float32))


# revision 33
# speedup vs baseline: 3.9534x; 3.9534x over previous
"""Trainium2 Bass kernel for top-2 MoE routing (nn_JaxMoE_26431228740246).

Strategy: F-parallel over all experts across 8 NeuronCores with host-side
routing/dispatch (the standard MoE dispatch/combine).  The router is
T*D*E = 16.8M MACs -- 0.008% of total FLOPs -- and determines the sharding,
so it runs on host.  Tokens are gathered per expert (token t appears in the
segments of its top-2 experts; Sum n_e = T*K = 4096 exactly); every core
receives the SAME dispatched token buffer but only an F/8 = 512 slice of
every expert's gate/up/down weights.  Each core computes, for each expert
segment, h = silu(Wg_slice.T x)*(Wu_slice.T x) and the partial down
projection y_c = Wd_slice.T h.  Host combine sums the 8 partial outputs and
scatter-adds with the renormalized router weights.

This is perfectly load balanced (all cores do identical 96*4096 = 393k PE
cycles -- the exact useful-FLOP floor at full-rate matmul) with zero
capacity padding, because bf16 matmuls run 1 cycle/row at any free-dim
width.

Everything on-device is bf16 (inputs quantized on host; PSUM accumulation is
fp32; partial outputs returned bf16 and summed in fp32 on host) -- measured
end-to-end error ~6e-4 against the fp32 reference, tolerance is 2e-2.

DMA layout notes: all tiles are host-pre-tiled so each DMA moves multi-KB
contiguous per-partition lines.  Gate/up weights + x stream on the sync
HWDGE queue; down weights + outputs use the scalar queue (separate FIFO +
second descriptor generator) so they prefetch without head-of-line blocking.

Shapes (hardcoded): T=2048, D=1024, F=4096, E=8, K=2 (top-k renormalized).
"""

import os
import sys

import numpy as np


def _ensure_path():
    for p in (
        "/root/.axon_site",
        "/root/.axon_site/_ro/trn_rl_repo",
        "/root/.axon_site/_ro/pypackages",
        "/opt/trn_rl_repo",
    ):
        if os.path.isdir(p) and p not in sys.path:
            sys.path.append(p)


_ensure_path()

T, D, F, E = 2048, 1024, 4096, 8
DT = D // 128    # 8 d-tiles
FS = F // E      # 512 = per-core F slice
FST = FS // 128  # 4 f-tiles per expert per core

_CACHE = {}


def _chunks(n):
    """Split a segment of n tokens into <=512-wide PSUM chunks (even halves)."""
    if n <= 512:
        return [(0, n)]
    h = (n + 1) // 2
    return [(0, h), (h, n)]


def _build(counts):
    import concourse.tile as tile
    from concourse import bacc, mybir

    fp32 = mybir.dt.float32
    bf16 = mybir.dt.bfloat16
    Act = mybir.ActivationFunctionType

    counts = list(counts)
    N = sum(counts)
    off = np.concatenate([[0], np.cumsum(counts)]).astype(int)

    nc = bacc.Bacc("TRN2", target_bir_lowering=False, debug=False, num_devices=E)

    # x dispatched, transposed, per-expert-contiguous: [128(di), sum_e 8*n_e]
    xT_d = nc.dram_tensor("xT", [128, DT * N], bf16, kind="ExternalInput").ap()
    # combined weight tile per (expert, f-tile): [128, {wg(do,f)|wu(do,f)|wd(dt,dc)}]
    # wd rides the gate-weight stream so its DMAs are paced by the pool
    # rotation instead of being hoisted to t=0 by the scheduler.
    w_d = nc.dram_tensor(
        "w", [E, FST, 128, 3, DT, 128], bf16, kind="ExternalInput"
    ).ap()
    # partial output: per expert a [128, DT, n_e] block, flattened columns
    out_d = nc.dram_tensor("out", [128, DT * N], bf16, kind="ExternalOutput").ap()

    from contextlib import ExitStack

    with tile.TileContext(nc) as tc, ExitStack() as ctx:
        px = ctx.enter_context(tc.tile_pool(name="x", bufs=1))
        pw = ctx.enter_context(tc.tile_pool(name="w", bufs=9))
        ph = ctx.enter_context(tc.tile_pool(name="h", bufs=1))
        ptmp = ctx.enter_context(tc.tile_pool(name="tmp", bufs=4))
        pout = ctx.enter_context(tc.tile_pool(name="out", bufs=3))
        pmm = ctx.enter_context(tc.tile_pool(name="mm", bufs=8, space="PSUM"))

        # ---- per expert: gate/up -> h, then partial down-projection ----
        # (down(e) overlaps gate(e+1) weight streaming; out DMAs spread
        # across the whole kernel instead of bursting at the end)
        for e in range(E):
            n = counts[e]
            ch = _chunks(n)
            xsrc = xT_d[:, DT * off[e] : DT * off[e + 1]].rearrange(
                "p (do n) -> p do n", do=DT
            )
            if e == 0:
                # separate per-do tiles: dependency tracking is per-tile, so
                # the first accumulation chain streams in do-granular pieces
                xe = [px.tile([128, n], bf16, tag=f"x0_{do}", name=f"x0_{do}") for do in range(DT)]
            else:
                xe = px.tile([128, DT, n], bf16, tag=f"x{e}")
            he = ph.tile([128, FST, n], bf16, tag=f"h{e}")

            def xop(do, c0, c1):
                if e == 0:
                    return xe[do][:, c0:c1]
                return xe[:, do, c0:c1]

            # gate/up phase for this expert
            wts = []
            for ft in range(FST):
                wt = pw.tile([128, 3, DT, 128], bf16, tag="w", name=f"w{e}_{ft}")
                if e == 0 and ft == 0:
                    # startup: interleave the first weight tile's three parts
                    # with the per-do x pieces so the first accumulation
                    # chain's operands land in consumption order
                    nc.sync.dma_start(wt[:, 0], w_d[e, ft, :, 0])
                    nc.sync.dma_start(xe[0][:], xsrc[:, 0, :])
                    nc.sync.dma_start(xe[1][:], xsrc[:, 1, :])
                    nc.sync.dma_start(wt[:, 1], w_d[e, ft, :, 1])
                    for do in range(2, DT):
                        nc.sync.dma_start(xe[do][:], xsrc[:, do, :])
                    nc.sync.dma_start(wt[:, 2], w_d[e, ft, :, 2])
                else:
                    nc.sync.dma_start(wt[:], w_d[e, ft])
                    if ft == 0 and e > 0:
                        nc.sync.dma_start(xe[:], xsrc)
                wts.append(wt)

                pgs = [pmm.tile([128, c1 - c0], fp32, tag="mm", name="pg") for c0, c1 in ch]
                pus = [pmm.tile([128, c1 - c0], fp32, tag="mm", name="pu") for c0, c1 in ch]
                for (c0, c1), pg, pu in zip(ch, pgs, pus):
                    for do in range(DT):
                        nc.tensor.matmul(
                            pg[:], wt[:, 0, do, :], xop(do, c0, c1),
                            start=(do == 0), stop=(do == DT - 1),
                        )
                    for do in range(DT):
                        nc.tensor.matmul(
                            pu[:], wt[:, 1, do, :], xop(do, c0, c1),
                            start=(do == 0), stop=(do == DT - 1),
                        )
                    tmp = ptmp.tile([128, c1 - c0], fp32, tag="tmp")
                    nc.scalar.activation(tmp[:], pg[:], Act.Silu)
                    nc.vector.tensor_mul(he[:, ft, c0:c1], tmp[:], pu[:])

            # down phase for this expert (wd slice fo=ft lives in wts[ft])
            chd = ch
            ot = pout.tile([128, DT, n], bf16, tag="ot")
            for dd in range(DT):
                for ci, (c0, c1) in enumerate(chd):
                    po = pmm.tile([128, c1 - c0], fp32, tag="mm")
                    for fo in range(FST):
                        nc.tensor.matmul(
                            po[:], wts[fo][:, 2, dd, :],
                            he[:, fo, c0:c1],
                            start=(fo == 0), stop=(fo == FST - 1),
                        )
                    # alternate copy engine: DVE / Activation (last expert:
                    # DVE only, so the Act SEQ stays free for tail DMAs)
                    if e == E - 1 or (dd * len(ch) + ci) % 2 == 0:
                        nc.vector.tensor_copy(ot[:, dd, c0:c1], po[:])
                    else:
                        nc.scalar.copy(ot[:, dd, c0:c1], po[:])
                if dd == DT // 2 - 1:
                    nc.scalar.dma_start(
                        out_d[:, DT * off[e] : DT * off[e] + (DT // 2) * n],
                        ot[:, 0 : DT // 2, :],
                    )
                elif e == E - 1 and dd >= DT // 2:
                    # last expert: ship each dd as soon as it's copied,
                    # spread across queues so descriptor generation overlaps
                    # and the final drain is one small DMA on an idle queue
                    q = [nc.scalar, nc.sync, nc.scalar, nc.sync][dd - DT // 2]
                    q.dma_start(
                        out_d[:, DT * off[e] + dd * n : DT * off[e] + (dd + 1) * n],
                        ot[:, dd, :],
                    )
            if e != E - 1:
                nc.scalar.dma_start(
                    out_d[:, DT * off[e] + (DT // 2) * n : DT * off[e + 1]],
                    ot[:, DT // 2 : DT, :],
                )

    nc.compile()
    return nc


def _get_nc(counts):
    key = tuple(counts)
    if key not in _CACHE:
        _CACHE[key] = _build(key)
    return _CACHE[key]


def _route(x, wr):
    """Exact top-2 routing in fp64 (verified: gap between 2nd/3rd router
    logit is ~5e-4 on this data, far above fp32 matmul noise, so fp64
    ordering equals the reference's fp32 ordering)."""
    lg = x.astype(np.float64) @ wr.astype(np.float64)           # [T, E]
    top2 = np.argpartition(-lg, 2, axis=1)[:, :2]               # unordered top-2
    l2 = np.take_along_axis(lg, top2, axis=1)                   # [T, 2]
    m = l2.max(axis=1, keepdims=True)
    p = np.exp(l2 - m)
    w2 = p / p.sum(axis=1, keepdims=True)                       # renormalized
    return top2, w2


def kernel(
    x_TD, w_router_DE, kernel_gating_EDF, kernel_up_proj_EDF, kernel_down_proj_EFD
):
    import ml_dtypes
    from concourse.bass_utils import run_bass_kernel_spmd

    bf = ml_dtypes.bfloat16

    x = np.ascontiguousarray(np.asarray(x_TD, dtype=np.float32))
    wr = np.ascontiguousarray(np.asarray(w_router_DE, dtype=np.float32))
    g = np.asarray(kernel_gating_EDF, dtype=np.float32)
    u = np.asarray(kernel_up_proj_EDF, dtype=np.float32)
    d = np.asarray(kernel_down_proj_EFD, dtype=np.float32)

    # ---- host routing / dispatch ----
    top2, w2 = _route(x, wr)
    idx_list, wgt_list = [], []
    for e in range(E):
        sel = top2 == e                                         # [T, 2]
        m = sel.any(axis=1)
        tok = np.nonzero(m)[0]
        idx_list.append(tok)
        wgt_list.append(w2[m][sel[m]].astype(np.float64))
    counts = [len(i) for i in idx_list]
    N = sum(counts)
    off = np.concatenate([[0], np.cumsum(counts)]).astype(int)

    # dispatched tokens, transposed, per-expert contiguous [128, sum_e 8*n_e]
    xT_host = np.empty((128, DT * N), dtype=bf)
    for e in range(E):
        seg = x[idx_list[e]].T.reshape(DT, 128, counts[e]).transpose(1, 0, 2)
        xT_host[:, DT * off[e] : DT * off[e + 1]] = seg.reshape(
            128, DT * counts[e]
        ).astype(bf)

    nc = _get_nc(counts)

    # per-core weight slices (core c takes F columns [c*512, (c+1)*512))
    in_maps = []
    for c in range(E):
        fs = slice(c * FS, (c + 1) * FS)
        # [E, D, FS] -> [E, FST, 128di, DT, 128f]
        wg_host = g[:, :, fs].reshape(E, DT, 128, FST, 128).transpose(0, 3, 2, 1, 4)
        wu_host = u[:, :, fs].reshape(E, DT, 128, FST, 128).transpose(0, 3, 2, 1, 4)
        # [E, FS, D] -> [E, FST, 128fi, DT, 128dc]
        wd_host = d[:, fs, :].reshape(E, FST, 128, DT, 128)
        w_host = np.ascontiguousarray(
            np.stack([wg_host, wu_host, wd_host], axis=3)
        ).astype(bf)                       # [E, FST, 128, 3, DT, 128]
        in_maps.append({"xT": xT_host, "w": w_host})

    trace = bool(os.environ.get("BASS_PROF"))
    try:
        res = run_bass_kernel_spmd(nc, in_maps, list(range(E)), trace=trace)
    except Exception:
        if not trace:
            raise
        res = run_bass_kernel_spmd(nc, in_maps, list(range(E)), trace=False)
    _CACHE["last_result"] = res
    _CACHE["last_counts"] = counts

    # ---- host combine: sum partial outputs, scatter-add with router weights ----
    ysum = np.zeros((128, DT * N), dtype=np.float32)
    for c in range(E):
        ysum += np.asarray(res.results[c]["out"], dtype=np.float32).reshape(
            128, DT * N
        )
    out = np.zeros((T, D), dtype=np.float64)
    for e in range(E):
        n = counts[e]
        blk = ysum[:, DT * off[e] : DT * off[e + 1]].reshape(128, DT, n)
        y = blk.transpose(1, 0, 2).reshape(D, n)                # [D, n]
        out[idx_list[e]] += wgt_list[e][:, None] * y.T
    return np.ascontiguousarray(out.astype(np.float32))
